# revision 21
# baseline (speedup 1.0000x reference)
"""Dilated attention (segment 64, dilation 4, 16 heads, head_dim 64) on 8 trn2 cores.

Sharding: 2 batches x 4 head-groups (4 heads each) = 8 cores. Each core computes
q/k/v projections for its 4 heads on its batch, block-sparse attention over the
+-2 block (256-token) dilated band, and a partial output projection. Host sums
the 4 head-group partials per batch.

Layout is fully "transposed" on-core to avoid PE transposes:
  xT   [D, S]    (D on partitions, 8 chunks of 128)
  qT/kT [64h, S] per head (head dim on partitions)
  v    [S, 64]   natural (keys on partitions) + ones column -> softmax denoms
  scoresT [k-block 128, q-window <=640] = kT_blk-stationary x qT-window

v2 changes vs baseline:
  - input DMAs spread over 4 queues (scalar/sync/gpsimd/vector) with
    x + pair-0 weight halves prioritized; k0-proj moved to the pre-era
    (dc-outer accumulators alongside q0) so attention starts ~15us in.
  - scores for both heads of a pair go into one 4-bank PSUM tile
    [128, 2, 1024]; the two heads' MMs occupy disjoint PE row groups
    (rows 0-63 / 64-127) and run concurrently; ONE merged exp per
    key-block [128, 2, 640] halves ACT per-call overhead; one mask
    multiply with an hh-broadcast AP.
  - PV accumulates into [65, 512] quarter tiles (1 bank each, 3-buf
    pool); per-quarter normalization (reciprocal reads PSUM directly,
    no ACT den copy) shortens the serial tail chain.
  - v-proj copies on ACT; q/k proj copies alternate DVE/ACT.
  - y projection gets its own 4-bank PSUM pool after the scores pool
    closes; psum->bf16 casts split across ACT/DVE; output DMA is bf16
    (4MB/core) spread over 3 queues. Host sums partials in f32.
"""

import numpy as np
import ml_dtypes

bfloat16 = ml_dtypes.bfloat16

B, S, D = 2, 2048, 1024
H, Dh = 16, 64
NCORES = 8
NKB = S // 128  # 16 key blocks
WMAX = 640

_cache = {}


def _mask_rel():
    kp = np.arange(128)[:, None]
    j = np.arange(WMAX)[None, :]
    qrel = j - 256
    diff = np.abs(qrel - kp)
    seg = (qrel // 64) == (kp // 64)
    dil = (diff > 0) & (diff % 4 == 0) & (diff <= 256)
    return np.ascontiguousarray((seg | dil).astype(bfloat16))


def _win(kb):
    return max(0, kb - 2) * 128, min(NKB, kb + 3) * 128


def _pv_pieces(kb):
    """Split PV matmul for key-block kb at 512-query (=1 psum bank) quarters.

    PSUM start_tensor_calc zeroes the whole 2KB bank, so start/stop are
    per QUARTER: the first key-block whose window touches a quarter opens
    it (start=True zeroes it), the last closes it."""
    q0, q1 = _win(kb)
    out = []
    a = q0
    while a < q1:
        b = min(q1, (a // 512 + 1) * 512)
        qt = a // 512
        st = kb == max(0, 4 * qt - 2)
        sp = kb == min(NKB - 1, 4 * qt + 5)
        out.append((a, b, qt, st, sp))
        a = b
    return q0, q1, out


def _build(debug=False):
    key = ("nc", debug)
    if key in _cache:
        return _cache[key]
    import concourse.mybir as mybir
    from concourse import bacc
    from concourse.tile import TileContext

    bf = mybir.dt.bfloat16
    f32 = mybir.dt.float32
    EXP = mybir.ActivationFunctionType.Exp

    nc = bacc.Bacc()
    d_x = nc.declare_dram_parameter("xT", [128, 8, S], bf, isOutput=False)
    d_wq0 = nc.declare_dram_parameter("wq0", [128, 8, 128], bf, isOutput=False)
    d_wq1 = nc.declare_dram_parameter("wq1", [128, 8, 128], bf, isOutput=False)
    d_wk0 = nc.declare_dram_parameter("wk0", [128, 8, 128], bf, isOutput=False)
    d_wk1 = nc.declare_dram_parameter("wk1", [128, 8, 128], bf, isOutput=False)
    d_wv = nc.declare_dram_parameter("wv", [128, 8, 256], bf, isOutput=False)
    d_wo = nc.declare_dram_parameter("wo", [128, 2, 1024], bf, isOutput=False)
    d_mask = nc.declare_dram_parameter("maskT", [128, WMAX], bf, isOutput=False)
    d_y = nc.declare_dram_parameter("yT", [128, 8, S], bf, isOutput=True)
    if debug:
        d_dbg_q = nc.declare_dram_parameter("dbg_q", [128, S], bf, isOutput=True)
        d_dbg_k = nc.declare_dram_parameter("dbg_k", [128, S], bf, isOutput=True)
        d_dbg_v = nc.declare_dram_parameter("dbg_v", [128, 16 * 4 * 65], bf, isOutput=True)
        d_dbg_at = nc.declare_dram_parameter("dbg_at", [128, WMAX], bf, isOutput=True)
        d_dbg_on = nc.declare_dram_parameter("dbg_on", [128, S], bf, isOutput=True)

    with TileContext(nc) as tc:
        with (
            tc.tile_pool(name="const", bufs=1) as cpool,
            tc.tile_pool(name="attn", bufs=30) as apool,
            tc.tile_pool(name="ysb", bufs=6) as ypool,
            tc.tile_pool(name="small", bufs=6) as spool,
        ):
            # ---- input DMAs: 4 queues; x + pair-0 weights first ----
            sb_wq = cpool.tile([128, 8, 2, 128], bf, name="wq", tag="wq")
            sb_wk = cpool.tile([128, 8, 2, 128], bf, name="wk", tag="wk")
            sb_wv = cpool.tile([128, 8, 256], bf, name="wv", tag="wv")
            sb_wo = cpool.tile([128, 2, 1024], bf, name="wo", tag="wo")
            sb_mask = cpool.tile([128, WMAX], bf, name="mask", tag="mask")
            sb_xall = cpool.tile([128, 8, S], bf, name="xall", tag="xall")
            sb_x = [sb_xall[:, dc, :] for dc in range(8)]

            junk = cpool.tile([128, 256], bf, name="junk", tag="junk")
            nc.gpsimd.memset(junk, 0.0)

            # x has priority: tiny pair-0 weights first, then all of x in
            # dc-prefix order round-robin, then the late-needed weights.
            nc.scalar.dma_start(out=sb_wq[:, :, 0, :], in_=d_wq0[:, :, :])
            nc.sync.dma_start(out=sb_wk[:, :, 0, :], in_=d_wk0[:, :, :])
            nc.gpsimd.dma_start(out=sb_mask, in_=d_mask[:, :])
            for tt in range(4):
                sl = slice(tt * 512, (tt + 1) * 512)
                nc.scalar.dma_start(
                    out=sb_xall[:, 0:3, sl], in_=d_x[:, 0:3, sl])
                nc.sync.dma_start(
                    out=sb_xall[:, 3:6, sl], in_=d_x[:, 3:6, sl])
                nc.gpsimd.dma_start(
                    out=sb_xall[:, 6:8, sl], in_=d_x[:, 6:8, sl])
            nc.scalar.dma_start(out=sb_wq[:, :, 1, :], in_=d_wq1[:, :, :])
            nc.sync.dma_start(out=sb_wk[:, :, 1, :], in_=d_wk1[:, :, :])
            nc.gpsimd.dma_start(out=sb_wv[:, :, :], in_=d_wv[:, :, :])
            nc.sync.dma_start(out=sb_wo[:, 0, :], in_=d_wo[:, 0, :])
            nc.gpsimd.dma_start(out=sb_wo[:, 1, :], in_=d_wo[:, 1, :])

            sb_q = []
            sb_k = []
            sb_on = []
            for p in range(2):
                sb_q.append(cpool.tile([128, S], bf, name=f"q{p}", tag=f"q{p}"))
                sb_k.append(cpool.tile([128, S], bf, name=f"k{p}", tag=f"k{p}"))
                sb_on.append(cpool.tile([128, S], bf, name=f"on{p}", tag=f"on{p}"))
            sb_v = cpool.tile([128, 16, 4, 65], bf, name="v", tag="v")
            nc.vector.memset(sb_v[:, :, :, 64:65], 1.0)

            # ---- attention era ----
            with tc.tile_pool(name="ot", bufs=3, space="PSUM") as otp:

                def scores_range(p, scp, ats, kb_lo, kb_hi):
                    with nc.named_scope(f"scores_p{p}"), tc.high_priority():
                        for kb in range(kb_lo, kb_hi + 1):
                            q0, q1 = _win(kb)
                            wk_ = q1 - q0
                            j0 = q0 - (kb - 2) * 128
                            sc = scp.tile([128, 2, 1024], f32, name="sc", tag="sc")
                            pieces = [(0, min(wk_, 512))]
                            if wk_ > 512:
                                pieces.append((512, wk_))
                            for a, b in pieces:
                                for hh in range(2):
                                    half = hh * 64
                                    nc.tensor.matmul(
                                        sc[:, hh, a:b],
                                        lhsT=sb_k[p][half:half + 64, kb * 128:(kb + 1) * 128],
                                        rhs=sb_q[p][half:half + 64, q0 + a:q0 + b],
                                        start=True,
                                        stop=True,
                                    )
                            at = apool.tile([128, 2, WMAX], bf, name="at", tag="at")
                            nc.scalar.activation(at[:, :, :wk_], sc[:, :, :wk_], EXP)
                            mk = sb_mask[:, j0:j0 + wk_].rearrange(
                                "p (o w) -> p o w", o=1).broadcast_to([128, 2, wk_])
                            nc.vector.tensor_mul(at[:, :, :wk_], at[:, :, :wk_], mk)
                            ats[kb] = at
                            if debug and p == 0 and kb == 8:
                                nc.sync.dma_start(out=d_dbg_at[:, :], in_=at[:, 0, :])

                def scores_phase(p, scp):
                    ats = {}
                    scores_range(p, scp, ats, 0, NKB - 1)
                    return ats

                def proj_qk(w_sb, dst, p, scope, pj, eng_alt):
                    with nc.named_scope(scope):
                        for tt in range(4):
                            ps = pj.tile([128, 512], f32, name="pspj", tag="pj")
                            for dc in range(8):
                                nc.tensor.matmul(
                                    ps,
                                    lhsT=w_sb[:, dc, p, :],
                                    rhs=sb_x[dc][:, tt * 512:(tt + 1) * 512],
                                    start=(dc == 0),
                                    stop=(dc == 7),
                                )
                            if (tt + eng_alt) % 2 == 0:
                                nc.vector.tensor_copy(dst[p][:, tt * 512:(tt + 1) * 512], ps)
                            else:
                                nc.scalar.copy(dst[p][:, tt * 512:(tt + 1) * 512], ps)

                def proj_v(pj):
                    with nc.named_scope("proj_v"):
                        for t in range(16):
                            ps = pj.tile([128, 256], f32, name="psv", tag="pj")
                            for dc in range(8):
                                nc.tensor.matmul(
                                    ps,
                                    lhsT=sb_x[dc][:, t * 128:(t + 1) * 128],
                                    rhs=sb_wv[:, dc, :],
                                    start=(dc == 0),
                                    stop=(dc == 7),
                                )
                            if t % 2 == 0:
                                nc.scalar.copy(
                                    sb_v[:, t, :, 0:64],
                                    ps.rearrange("p (h d) -> p h d", h=4),
                                )
                            else:
                                nc.vector.tensor_copy(
                                    sb_v[:, t, :, 0:64],
                                    ps.rearrange("p (h d) -> p h d", h=4),
                                )

                STAGES = [(0, 5), (6, 9), (10, 13), (14, 15)]

                def pv_stage(p, hh, ats, outq, kb_lo, kb_hi):
                    h = 2 * p + hh
                    half = hh * 64
                    with nc.named_scope(f"pv_h{h}s{kb_lo}"):

                        def normalize(qt):
                            base = qt * 512
                            with tc.high_priority():
                                den = spool.tile([1, 512], f32, name="den", tag="den")
                                nc.scalar.copy(den, outq[qt][64:65, :])
                                rec = spool.tile([1, 512], f32, name="rec", tag="rec")
                                nc.vector.reciprocal_approx_fast(rec, den)
                                bc = spool.tile([64, 512], f32, name="bc", tag="bc")
                                nc.gpsimd.partition_broadcast(bc, rec)
                                nc.vector.tensor_mul(
                                    sb_on[p][half:half + 64, base:base + 512],
                                    outq[qt][0:64, :], bc,
                                )

                        for kb in range(kb_lo, kb_hi + 1):
                            q0, q1, pieces = _pv_pieces(kb)
                            at = ats[kb]
                            vv = sb_v[:, kb, h, :]
                            closed = []
                            for a, b, qt, st, sp_ in pieces:
                                if st:
                                    outq[qt] = otp.tile([65, 512], f32,
                                                        name=f"o{h}{qt}", tag="outp")
                                nc.tensor.matmul(
                                    outq[qt][:, a - qt * 512:b - qt * 512],
                                    lhsT=vv,
                                    rhs=at[:, hh, a - q0:b - q0],
                                    start=st,
                                    stop=sp_,
                                )
                                if sp_:
                                    closed.append(qt)
                            for qt in closed:
                                normalize(qt)

                # pair 0 attention; the q0/k0 projection chains recycle ONE
                # psum bank and are emitted interleaved with the scores-0 kb
                # groups so the exp pipeline starts as soon as x-tt0 lands.
                with tc.tile_pool(name="sc", bufs=1, space="PSUM") as scp:
                    ats0 = {}
                    with tc.tile_pool(name="pre", bufs=1, space="PSUM") as prep:
                        jt = prep.tile([128, 512], f32, name="acc", tag="acc")
                        with nc.named_scope("warmup"):
                            for i in range(48):
                                nc.tensor.matmul(jt[:, 0:256], lhsT=junk[:, 0:128],
                                                 rhs=junk, start=True, stop=True)
                        GROUPS = [(0, 1), (2, 5), (6, 9), (10, 15)]
                        for tt in range(4):
                            sl = slice(tt * 512, (tt + 1) * 512)
                            with nc.named_scope(f"proj_qk0_{tt}"):
                                for w_sb, dst in ((sb_wq, sb_q), (sb_wk, sb_k)):
                                    acc = prep.tile([128, 512], f32, name="acc", tag="acc")
                                    for dc in range(8):
                                        nc.tensor.matmul(
                                            acc,
                                            lhsT=w_sb[:, dc, 0, :],
                                            rhs=sb_x[dc][:, sl],
                                            start=(dc == 0),
                                            stop=(dc == 7),
                                        )
                                    nc.vector.tensor_copy(dst[0][:, sl], acc)
                            scores_range(0, scp, ats0, *GROUPS[tt])
                    with tc.tile_pool(name="pj", bufs=1, space="PSUM") as pj:
                        proj_qk(sb_wq, sb_q, 1, "proj_q1", pj, 0)
                        proj_qk(sb_wk, sb_k, 1, "proj_k1", pj, 1)
                        ats1 = scores_phase(1, scp)
                        proj_v(pj)
                        outq0 = {0: {}, 1: {}}
                        for lo, hi in STAGES:
                            pv_stage(0, 0, ats0, outq0[0], lo, hi)
                            pv_stage(0, 1, ats0, outq0[1], lo, hi)
                        outq1 = {0: {}, 1: {}}
                        for lo, hi in STAGES:
                            pv_stage(1, 0, ats1, outq1[0], lo, hi)
                            pv_stage(1, 1, ats1, outq1[1], lo, hi)

                if debug:
                    nc.sync.dma_start(out=d_dbg_q[:, :], in_=sb_q[0][:, :])
                    nc.sync.dma_start(out=d_dbg_k[:, :], in_=sb_k[0][:, :])
                    nc.sync.dma_start(
                        out=d_dbg_v[:, :],
                        in_=sb_v.rearrange("p a b c -> p (a b c)"),
                    )

                # ---- pair-1 PV + output projection: yT = wo^T @ outT_norm ----
                with tc.tile_pool(name="ypsum", bufs=2, space="PSUM") as yps:
                    if debug:
                        nc.sync.dma_start(out=d_dbg_on[:, :], in_=sb_on[0][:, :])

                    with nc.named_scope("proj_y"):
                        qdma = [nc.sync, nc.gpsimd]
                        for tt in range(4):
                            for dch in range(4):
                                ysb = ypool.tile([128, 1024], bf, name="ysb", tag="ysb")
                                ps = yps.tile([128, 1024], f32, name="psy", tag="psy")
                                for sub in range(2):
                                    dc = dch * 2 + sub
                                    for kc in range(2):
                                        nc.tensor.matmul(
                                            ps[:, sub * 512:(sub + 1) * 512],
                                            lhsT=sb_wo[:, kc, dc * 128:(dc + 1) * 128],
                                            rhs=sb_on[kc][:, tt * 512:(tt + 1) * 512],
                                            start=(kc == 0),
                                            stop=(kc == 1),
                                        )
                                if (tt * 4 + dch) % 2 == 0:
                                    nc.scalar.copy(ysb, ps)
                                else:
                                    nc.vector.tensor_copy(ysb, ps)
                                if tt == 3:
                                    q3 = [nc.sync, nc.gpsimd, nc.scalar]
                                    for c in range(2):
                                        q3[(dch * 2 + c) % 3].dma_start(
                                            out=d_y[:, dch * 2 + c, tt * 512:(tt + 1) * 512],
                                            in_=ysb[:, c * 512:(c + 1) * 512],
                                        )
                                else:
                                    eng = qdma[(tt * 4 + dch) % 2]
                                    eng.dma_start(
                                        out=d_y[:, dch * 2:dch * 2 + 2, tt * 512:(tt + 1) * 512],
                                        in_=ysb.rearrange("p (c t) -> p c t", c=2),
                                    )

    nc.compile()
    _cache[key] = nc
    return nc


def kernel(hidden_states, w_q, w_k, w_v, w_o, _debug=False):
    from concourse.bass_utils import run_bass_kernel_spmd

    nc = _build(debug=_debug)
    mask = _mask_rel()
    scale = np.float32(Dh ** -0.5)

    def chunk_dmajor(w, rows, cols):
        return np.ascontiguousarray(
            w.reshape(rows, 128, cols).transpose(1, 0, 2)
        )

    in_maps = []
    for c in range(NCORES):
        b, hg = c // 4, c % 4
        hsl = slice(hg * 256, (hg + 1) * 256)
        xT = np.asarray(hidden_states[b]).T.astype(bfloat16)  # [D, S]
        wq = chunk_dmajor((np.asarray(w_q[:, hsl]) * scale).astype(bfloat16), 8, 256)
        wk = chunk_dmajor(np.asarray(w_k[:, hsl]).astype(bfloat16), 8, 256)
        in_maps.append({
            "xT": chunk_dmajor(xT, 8, S),
            "wq0": np.ascontiguousarray(wq[:, :, 0:128]),
            "wq1": np.ascontiguousarray(wq[:, :, 128:256]),
            "wk0": np.ascontiguousarray(wk[:, :, 0:128]),
            "wk1": np.ascontiguousarray(wk[:, :, 128:256]),
            "wv": chunk_dmajor(np.asarray(w_v[:, hsl]).astype(bfloat16), 8, 256),
            "wo": chunk_dmajor(np.asarray(w_o[hsl, :]).astype(bfloat16), 2, 1024),
            "maskT": mask,
        })

    res = run_bass_kernel_spmd(nc, in_maps, list(range(NCORES)))
    _cache["last_results"] = res

    y = np.zeros((B, S, D), np.float32)
    for c in range(NCORES):
        yT = np.asarray(res.results[c]["yT"], np.float32)  # [128, 8, S]
        y[c // 4] += yT.transpose(1, 0, 2).reshape(D, S).T
    return y
